# revision 4
# baseline (speedup 1.0000x reference)
"""Trainium2 Bass kernel for the AssociativeMemoryL1 problem.

out       = hidden + sigmoid(hidden @ Wg.T) * ((q@memory per head) @ Wo.T)
new_mem   = 0.99*memory + sum_tokens k^T v   (per head)

Strategy: data-parallel over the 16384 tokens across 8 NeuronCores
(2048 tokens/core), bf16 TensorEngine compute with fp32 PSUM
accumulation, host-side pre-transposed/bf16-staged operands, and an
on-device AllReduce of the per-core k^T v partial sums.
"""

import numpy as np
import ml_dtypes

import concourse.bacc as bacc
import concourse.mybir as mybir
import concourse.tile as tile
from concourse.bass_utils import run_bass_kernel_spmd

BF16 = mybir.dt.bfloat16
F32 = mybir.dt.float32
NPBF = ml_dtypes.bfloat16

N_CORES = 8
B, S, D = 4, 4096, 2048
H, DK, DV = 16, 64, 64
HD = H * DK  # 1024
T = (B * S) // N_CORES  # tokens per core = 2048
NT = T // 128  # 16 token tiles
ND = D // 128  # 16 contraction chunks
NTB = T // 512  # 4 512-token blocks
DECAY = 0.99

TRACE = False  # set True from test harness to capture HW exec time
LAST_RESULT = None  # BassKernelResults of the most recent run

_CACHE = {}


def _build():
    """Build + compile the 8-core SPMD graph once."""
    if "nc" in _CACHE:
        return _CACHE["nc"]

    nc = bacc.Bacc("TRN2", target_bir_lowering=False, debug=False,
                   num_devices=N_CORES)

    # ---- DRAM I/O (per-core shapes) ----
    xt_e = nc.dram_tensor("xt", [ND, 128, T], BF16, kind="ExternalInput")
    x_e = nc.dram_tensor("x", [T, D], F32, kind="ExternalInput")
    wkv_e = nc.dram_tensor("wkv", [ND, 128, 2048], BF16, kind="ExternalInput")
    wq_e = nc.dram_tensor("wq", [ND, 128, HD], BF16, kind="ExternalInput")
    wg_e = nc.dram_tensor("wg", [ND, 128, D], BF16, kind="ExternalInput")
    wo_e = nc.dram_tensor("wo", [HD // 128, 128, D], BF16, kind="ExternalInput")
    mem2_e = nc.dram_tensor("mem2", [128, HD], BF16, kind="ExternalInput")
    memp_e = nc.dram_tensor("memp", [128, 512], F32, kind="ExternalInput")

    out_e = nc.dram_tensor("out", [T, D], F32, kind="ExternalOutput")
    nm_e = nc.dram_tensor("newmem", [128, 512], F32, kind="ExternalOutput")

    rot_s = nc.dram_tensor("rot_scratch", [HD // 128, 128, T], BF16)
    ar_in = nc.dram_tensor("ar_in", [128, 512], F32)
    ar_out = nc.dram_tensor("ar_out", [128, 512], F32, addr_space="Shared")

    with tile.TileContext(nc) as tc:
        with (
            tc.tile_pool(name="xt", bufs=ND) as pxt,
            tc.tile_pool(name="const", bufs=1) as pconst,
        ):
            xts = []
            for dc in range(ND):
                t = pxt.tile([128, T], BF16, tag="xt")
                nc.sync.dma_start(t[:], xt_e[dc])
                xts.append(t)
            mem2 = pconst.tile([128, HD], BF16, tag="mem2")
            nc.sync.dma_start(mem2[:], mem2_e[:])

            # ================= P1: k,v projections + memory update ======
            with (
                tc.tile_pool(name="wkv", bufs=ND) as pw1,
                tc.tile_pool(name="kvsb", bufs=3) as pkv,
                tc.tile_pool(name="kvps", bufs=3, space="PSUM") as pp1,
                tc.tile_pool(name="memps", bufs=2, space="PSUM") as ppm,
            ):
                wkv = []
                for dc in range(ND):
                    t = pw1.tile([128, 2048], BF16, tag="wkv")
                    nc.sync.dma_start(t[:], wkv_e[dc])
                    wkv.append(t)
                # ping-pong fp32 SBUF accumulators for the k^T v partials
                macc = [
                    pconst.tile([128, 512], F32, tag="macc0", name="macc0"),
                    pconst.tile([128, 512], F32, tag="macc1", name="macc1"),
                ]
                for tt in range(NT):
                    kps = pp1.tile([128, 1024], F32, tag="kvps")
                    vps = pp1.tile([128, 1024], F32, tag="kvps")
                    for dc in range(ND):
                        lhs = xts[dc][:, tt * 128:(tt + 1) * 128]
                        st, sp = dc == 0, dc == ND - 1
                        nc.tensor.matmul(kps[:, 0:512], lhs,
                                         wkv[dc][:, 0:512], start=st, stop=sp)
                        nc.tensor.matmul(kps[:, 512:1024], lhs,
                                         wkv[dc][:, 512:1024], start=st, stop=sp)
                        nc.tensor.matmul(vps[:, 0:512], lhs,
                                         wkv[dc][:, 1024:1536], start=st, stop=sp)
                        nc.tensor.matmul(vps[:, 512:1024], lhs,
                                         wkv[dc][:, 1536:2048], start=st, stop=sp)
                    ksb = pkv.tile([128, 1024], BF16, tag="ksb")
                    vsb = pkv.tile([128, 1024], BF16, tag="vsb")
                    nc.vector.tensor_copy(ksb[:], kps[:])
                    nc.vector.tensor_copy(vsb[:], vps[:])
                    # Single-shot packed head outer products: each head
                    # region is written exactly once per token tile
                    # (start=True clears has-written bits for the whole
                    # bank row, so cross-tile PSUM accumulation of
                    # column-packed regions is unsafe).
                    mps = ppm.tile([128, 512], F32, tag="memps",
                                   name=f"memps{tt}")
                    for h in range(H):
                        po = (h % 2) * 64
                        fo = (h // 2) * 64
                        nc.tensor.matmul(
                            mps[po:po + 64, fo:fo + 64],
                            ksb[:, h * 64:(h + 1) * 64],
                            vsb[:, h * 64:(h + 1) * 64],
                            start=True, stop=True,
                            skip_group_check=True,
                        )
                    if tt == 0:
                        nc.vector.tensor_copy(macc[0][:], mps[:])
                    else:
                        nc.vector.tensor_add(macc[tt % 2][:],
                                             macc[(tt + 1) % 2][:], mps[:])
                msum = macc[(NT - 1) % 2]

            nc.sync.dma_start(ar_in[:], msum[:])
            nc.gpsimd.collective_compute(
                "AllReduce", mybir.AluOpType.add,
                ins=[ar_in[:]], outs=[ar_out[:]],
                replica_groups=[list(range(N_CORES))],
            )
            arsb = pconst.tile([128, 512], F32, tag="arsb")
            nc.sync.dma_start(arsb[:], ar_out[:])
            mpsb = pconst.tile([128, 512], F32, tag="mpsb")
            nc.sync.dma_start(mpsb[:], memp_e[:])
            dec = pconst.tile([128, 512], F32, tag="dec")
            nc.scalar.mul(dec[:], mpsb[:], DECAY)
            nmsb = pconst.tile([128, 512], F32, tag="nmsb")
            nc.vector.tensor_add(nmsb[:], dec[:], arsb[:])
            nc.sync.dma_start(nm_e[:], nmsb[:])

            # ================= P2: q^T + readout^T ======================
            with (
                tc.tile_pool(name="wq", bufs=ND) as pwq,
                tc.tile_pool(name="qsb", bufs=3) as pq,
                tc.tile_pool(name="rosb", bufs=3) as pro,
                tc.tile_pool(name="qps", bufs=5, space="PSUM") as pqps,
                tc.tile_pool(name="rops", bufs=2, space="PSUM") as props,
            ):
                wq = []
                for dc in range(ND):
                    t = pwq.tile([128, HD], BF16, tag="wq")
                    nc.sync.dma_start(t[:], wq_e[dc])
                    wq.append(t)
                for oc in range(HD // 128):
                    qps_l = [pqps.tile([128, 512], F32, tag="qps", name=f"qps{oc}_{tb}")
                             for tb in range(NTB)]
                    for dc in range(ND):
                        lhs = wq[dc][:, oc * 128:(oc + 1) * 128]
                        for tb in range(NTB):
                            nc.tensor.matmul(
                                qps_l[tb][:], lhs,
                                xts[dc][:, tb * 512:(tb + 1) * 512],
                                start=(dc == 0), stop=(dc == ND - 1))
                    ha, hb = 2 * oc, 2 * oc + 1
                    for tb in range(NTB):
                        qsb = pq.tile([128, 512], BF16, tag="qsb")
                        nc.vector.tensor_copy(qsb[:], qps_l[tb][:])
                        rops = props.tile([128, 512], F32, tag="rops")
                        nc.tensor.matmul(rops[0:64, :],
                                         mem2[0:64, ha * 64:(ha + 1) * 64],
                                         qsb[0:64, :], start=True, stop=True)
                        nc.tensor.matmul(rops[64:128, :],
                                         mem2[64:128, hb * 64:(hb + 1) * 64],
                                         qsb[64:128, :], start=True, stop=True)
                        rosb = pro.tile([128, 512], BF16, tag="rosb")
                        nc.vector.tensor_copy(rosb[:], rops[:])
                        nc.sync.dma_start(
                            rot_s[oc, :, tb * 512:(tb + 1) * 512], rosb[:])

            # ============ P3: gate + Wo projection + merge (2 halves) ===
            for half in range(2):
                cofs = half * 1024  # output-column offset
                with (
                    tc.tile_pool(name=f"wg{half}", bufs=ND) as pwg,
                    tc.tile_pool(name=f"wo{half}", bufs=HD // 128) as pwo,
                    tc.tile_pool(name=f"rob{half}", bufs=12) as prob,
                    tc.tile_pool(name=f"x3{half}", bufs=2) as px3,
                    tc.tile_pool(name=f"g3{half}", bufs=2) as pg3,
                    tc.tile_pool(name=f"m3{half}", bufs=2) as pm3,
                    tc.tile_pool(name=f"o3{half}", bufs=2) as po3,
                    tc.tile_pool(name=f"gps{half}", bufs=2, space="PSUM") as pgps,
                    tc.tile_pool(name=f"pps{half}", bufs=2, space="PSUM") as ppps,
                ):
                    wg = []
                    for dc in range(ND):
                        t = pwg.tile([128, 1024], BF16, tag=f"wg{half}")
                        nc.sync.dma_start(t[:], wg_e[dc, :, cofs:cofs + 1024])
                        wg.append(t)
                    wo = []
                    for hvc in range(HD // 128):
                        t = pwo.tile([128, 1024], BF16, tag=f"wo{half}")
                        nc.sync.dma_start(t[:], wo_e[hvc, :, cofs:cofs + 1024])
                        wo.append(t)
                    for tb in range(NTB):
                        robs = []
                        for hvc in range(HD // 128):
                            t = prob.tile([128, 512], BF16, tag=f"rob{half}")
                            nc.sync.dma_start(
                                t[:], rot_s[hvc, :, tb * 512:(tb + 1) * 512])
                            robs.append(t)
                        for ti in range(4):
                            tt = tb * 4 + ti
                            gps = pgps.tile([128, 1024], F32, tag=f"gps{half}")
                            for dc in range(ND):
                                lhs = xts[dc][:, tt * 128:(tt + 1) * 128]
                                st, sp = dc == 0, dc == ND - 1
                                nc.tensor.matmul(gps[:, 0:512], lhs,
                                                 wg[dc][:, 0:512],
                                                 start=st, stop=sp)
                                nc.tensor.matmul(gps[:, 512:1024], lhs,
                                                 wg[dc][:, 512:1024],
                                                 start=st, stop=sp)
                            gsb = pg3.tile([128, 1024], F32, tag=f"gsb{half}")
                            nc.scalar.activation(
                                gsb[:], gps[:],
                                mybir.ActivationFunctionType.Sigmoid)
                            pps = ppps.tile([128, 1024], F32, tag=f"pps{half}")
                            for hvc in range(HD // 128):
                                lhs = robs[hvc][:, ti * 128:(ti + 1) * 128]
                                st, sp = hvc == 0, hvc == HD // 128 - 1
                                nc.tensor.matmul(pps[:, 0:512], lhs,
                                                 wo[hvc][:, 0:512],
                                                 start=st, stop=sp)
                                nc.tensor.matmul(pps[:, 512:1024], lhs,
                                                 wo[hvc][:, 512:1024],
                                                 start=st, stop=sp)
                            xsb = px3.tile([128, 1024], F32, tag=f"xsb{half}")
                            nc.sync.dma_start(
                                xsb[:],
                                x_e[tt * 128:(tt + 1) * 128, cofs:cofs + 1024])
                            msb = pm3.tile([128, 1024], F32, tag=f"msb{half}")
                            nc.vector.tensor_mul(msb[:], gsb[:], pps[:])
                            osb = po3.tile([128, 1024], F32, tag=f"osb{half}")
                            nc.vector.tensor_add(osb[:], msb[:], xsb[:])
                            nc.sync.dma_start(
                                out_e[tt * 128:(tt + 1) * 128,
                                      cofs:cofs + 1024], osb[:])

    nc.compile()
    _CACHE["nc"] = nc
    return nc


def _stage(hidden, memory, Wk, Wv, Wq, Wg, Wo):
    """Host-side sharding + layout staging (bf16 casts, transposes)."""
    hs = np.ascontiguousarray(hidden.reshape(B * S, D))
    wkv = np.concatenate([Wk.T, Wv.T], axis=1).astype(NPBF).reshape(ND, 128, 2048)
    wq = Wq.T.astype(NPBF).reshape(ND, 128, HD)
    wg = Wg.T.astype(NPBF).reshape(ND, 128, D)
    wo = Wo.T.astype(NPBF).reshape(HD // 128, 128, D)
    memT = memory.transpose(1, 0, 2).reshape(DK, H * DV)
    mem2 = np.concatenate([memT, memT], axis=0).astype(NPBF)
    memp = np.zeros((128, 512), np.float32)
    for h in range(H):
        memp[(h % 2) * 64:(h % 2) * 64 + 64,
             (h // 2) * 64:(h // 2) * 64 + 64] = memory[h]

    in_maps = []
    for i in range(N_CORES):
        shard = hs[i * T:(i + 1) * T]
        xt = shard.T.astype(NPBF).reshape(ND, 128, T)
        in_maps.append({
            "xt": xt,
            "x": np.ascontiguousarray(shard, dtype=np.float32),
            "wkv": wkv, "wq": wq, "wg": wg, "wo": wo,
            "mem2": mem2, "memp": memp,
        })
    return in_maps


def kernel(hidden, memory, Wk, Wv, Wq, Wg, Wo):
    global LAST_RESULT
    nc = _build()
    in_maps = _stage(hidden, memory, Wk, Wv, Wq, Wg, Wo)

    kwargs = {}
    if TRACE:
        try:  # install NTFF profile hook if absent (best effort)
            import importlib.util
            import sys
            import types
            if "antenv.axon_hooks" not in sys.modules:
                spec = importlib.util.spec_from_file_location(
                    "_trn_boot", "/root/.axon_site/trn_agent_boot/trn_boot.py")
                boot = importlib.util.module_from_spec(spec)
                spec.loader.exec_module(boot)
                hook = boot._ntff_profile_via_ctypes("/opt/axon/libaxon_pjrt.so")
                mod = types.ModuleType("antenv.axon_hooks")
                mod._HOOK = hook
                mod.set_axon_ntff_profile_hook = lambda h: setattr(mod, "_HOOK", h)
                mod.get_axon_ntff_profile_hook = lambda: mod._HOOK
                sys.modules["antenv.axon_hooks"] = mod
                import antenv
                antenv.axon_hooks = mod
            kwargs["trace"] = True
        except Exception:
            pass

    res = run_bass_kernel_spmd(nc, in_maps, core_ids=list(range(N_CORES)),
                               **kwargs)
    LAST_RESULT = res

    out = np.concatenate([res.results[i]["out"] for i in range(N_CORES)],
                         axis=0).reshape(B, S, D)
    nm_p = res.results[0]["newmem"]
    new_memory = np.empty((H, DK, DV), np.float32)
    for h in range(H):
        new_memory[h] = nm_p[(h % 2) * 64:(h % 2) * 64 + 64,
                             (h // 2) * 64:(h // 2) * 64 + 64]
    return out, new_memory
